# revision 29
# baseline (speedup 1.0000x reference)
"""Trainium2 Bass kernel for nn_DensePoseV1ConvXGNInsHead:
2x (conv3x3 64->64 -> per-instance BN -> ReLU) on [8,64,256,256],
data-parallel one image per NeuronCore across 8 cores.

Self-contained: only imports the system concourse stack from /opt/trn_rl_repo.
"""
import os
import sys
import types

sys.path.insert(0, "/opt/trn_rl_repo")

import numpy as np

import concourse.bass as bass
import concourse.tile as tile
from concourse import mybir
from concourse.vector_clock import ScopedClock

f16 = mybir.dt.float16
f32 = mybir.dt.float32
ALU = mybir.AluOpType

C = 64          # channels
W = 256         # image width
PITCH = 272     # padded row pitch (16 left pad + 256 data; borrows next row's pad)
LP = 16         # left pad elements
R = 4           # conv rows per block (per half)
EPS = 1e-5

# ---------------------------------------------------------------------------
# walrus workaround: split the Tile exit-drain's sem waits (installed walrus
# rejects instructions with >2 sync waits)
# ---------------------------------------------------------------------------
_patched = False


def _install_tile_patch():
    global _patched
    if _patched:
        return
    _patched = True

    def _drain_and_barrier(self, tick_clock, wait_clock):
        nc = self.nc
        drain_inst = nc.sync.drain()
        wait_clock.add_sem_waits(
            drain_inst.ins, ScopedClock({None: tick_clock.global_clock})
        )
        si = drain_inst.ins.sync_info
        waits = list(si.on_wait or [])
        if len(waits) > 1:
            si.on_wait = waits[:1]
            for i in range(1, len(waits)):
                nop = nc.sync.nop()
                nop.ins.sync_info = mybir.SyncInfo(
                    on_wait=waits[i : i + 1], on_update=[]
                )
        nc.all_engine_barrier()
        popped = nc._tile_sem_poison_stack.pop()
        assert popped is self._sem_poison
        nc.clear_and_free_semaphores(list(self.sems.allocated().values()))
        nc.all_engine_barrier()

    tile.TileContext._drain_and_barrier = _drain_and_barrier


# ---------------------------------------------------------------------------
# NTFF profiling shim (antenv.axon_hooks is absent in this image)
# ---------------------------------------------------------------------------
def _install_ntff_shim():
    if "antenv.axon_hooks" in sys.modules:
        return
    mod = types.ModuleType("antenv.axon_hooks")
    state = {"hook": None}
    mod.set_axon_ntff_profile_hook = lambda h: state.__setitem__("hook", h)
    mod.get_axon_ntff_profile_hook = lambda: state["hook"]
    sys.modules["antenv.axon_hooks"] = mod
    try:
        import antenv

        antenv.axon_hooks = mod
    except ImportError:
        pass
    try:
        from trn_agent_boot.trn_boot import _ntff_profile_via_ctypes

        h = _ntff_profile_via_ctypes("/opt/axon/libaxon_pjrt.so")
        mod.set_axon_ntff_profile_hook(h)
    except Exception:
        pass


def yoff(slot):
    return slot * PITCH + LP


def _ap(base_ap, offset_elems, dims):
    """Build a sub-AP of base_ap at +offset (elements), with given free dims.

    base_ap must be a plain [P, F] tile AP; dims is a list of [step, count]
    free dims; partition dim is preserved."""
    return bass.AP(
        tensor=base_ap.tensor,
        offset=base_ap.offset + offset_elems,
        ap=[base_ap.ap[0]] + dims,
    )


def emit(nc, H):
    """Emit the full 2-layer kernel for an HxW image (H=256 in production)."""
    HH = H // 2
    NB = HH // R            # conv blocks per layer
    NST = HH * 2            # stats chunks (128 px each) per layer
    HW2 = HH * W
    NG = HH // 2            # normalize chunks (2 rows per half each) per layer
    assert HH % R == 0

    xh = nc.declare_dram_parameter("xh", [C, H * W], f16, isOutput=False)
    rcnt = nc.declare_dram_parameter("rcnt", [9], f32, isOutput=False)
    w0d = nc.declare_dram_parameter("w0d", [128, 9, 128], f16, isOutput=False)
    w1d = nc.declare_dram_parameter("w1d", [128, 9, 128], f16, isOutput=False)
    id128 = nc.declare_dram_parameter("id128", [128, 128], f16, isOutput=False)
    maskpm_d = nc.declare_dram_parameter("maskpm", [128, NST * 41], f16, isOutput=False)
    # +256-element row pad: row stride 66048B = 129*512B, coprime with the
    # 16-way DMA-engine interleave, so the load spreads across all engines
    ms2_d = nc.declare_dram_parameter("ms2", [128, HW2 + 256], f16, isOutput=False)
    g0 = nc.declare_dram_parameter("g0", [9, C], f32, isOutput=False)
    b0 = nc.declare_dram_parameter("b0", [9, C], f32, isOutput=False)
    g1 = nc.declare_dram_parameter("g1", [9, C], f32, isOutput=False)
    b1 = nc.declare_dram_parameter("b1", [9, C], f32, isOutput=False)
    bg9 = nc.declare_dram_parameter("bg9", [9, C], f32, isOutput=False)
    out = nc.declare_dram_parameter("out", [C, H * W], f16, isOutput=True)

    with tile.TileContext(nc) as tc:
        import contextlib

        with contextlib.ExitStack() as ctx:
            const = ctx.enter_context(tc.tile_pool(name="const", bufs=1))
            xbp = ctx.enter_context(tc.tile_pool(name="xbp", bufs=1))
            stripp = ctx.enter_context(tc.tile_pool(name="stripp", bufs=3))
            normp = ctx.enter_context(tc.tile_pool(name="normp", bufs=3))
            outp = ctx.enter_context(tc.tile_pool(name="outp", bufs=3))
            smallp = ctx.enter_context(tc.tile_pool(name="smallp", bufs=2))
            psc = ctx.enter_context(tc.tile_pool(name="psc", bufs=3, space="PSUM"))
            tpp = ctx.enter_context(tc.tile_pool(name="tpp", bufs=2, space="PSUM"))
            pss = ctx.enter_context(tc.tile_pool(name="pss", bufs=1, space="PSUM"))
            pse = ctx.enter_context(tc.tile_pool(name="pse", bufs=2, space="PSUM"))

            # ---- persistent buffers (only pads need zeroing)
            ysb = const.tile([128, (HH + 2) * PITCH + LP], f16)
            nc.vector.memset(_ap(ysb[:], 0, [[PITCH, HH + 3], [1, LP]]), 0.0)
            # top halo row for A (y row -1) and bottom halo row for B (y row H)
            nc.vector.memset(_ap(ysb[0:64, :], yoff(0), [[1, W]]), 0.0)
            nc.vector.memset(_ap(ysb[64:128, :], yoff(HH + 1), [[1, W]]), 0.0)
            xb0 = xbp.tile([128, (R + 2) * PITCH + LP], f16, tag="xb0")
            xb1 = xbp.tile([128, (R + 2) * PITCH + LP], f16, tag="xb1")
            nc.vector.memset(_ap(xb0[:], 0, [[PITCH, R + 3], [1, LP]]), 0.0)
            nc.vector.memset(_ap(xb1[:], 0, [[PITCH, R + 3], [1, LP]]), 0.0)
            xbs = [xb0, xb1]

            # ---- host-precomputed one-hot masks
            maskpm = const.tile([128, NST * 41], f16)
            ms2 = const.tile([128, HW2], f16)

            # ---- small constants
            id128sb = const.tile([128, 128], f16)
            nc.sync.dma_start(out=id128sb[:], in_=id128[:])
            rcsb = const.tile([9, 1], f32)
            nc.sync.dma_start(out=rcsb[:], in_=rcnt[:].rearrange("(a b) -> a b", b=1))
            epsap = const.tile([9, 1], f32)
            nc.vector.memset(epsap[:], EPS)
            gam = []
            bet = []
            for gg, bb in ((g0, b0), (g1, b1)):
                gt = const.tile([9, 64], f32, tag="gam")
                bt = const.tile([9, 64], f32, tag="bet")
                nc.sync.dma_start(out=gt[:], in_=gg[:])
                nc.sync.dma_start(out=bt[:], in_=bb[:])
                gam.append(gt)
                bet.append(bt)
            bg9sb = const.tile([9, 64], f32)
            nc.sync.dma_start(out=bg9sb[:], in_=bg9[:])
            wts = []
            for wd in (w0d, w1d):
                wt = const.tile([128, 9, 128], f16, tag="wt")
                nc.sync.dma_start(out=wt[:], in_=wd[:])
                wts.append(wt)

            slot0s = (1, 0)     # layer L writes y row r at slot r + slot0s[L]
            stats_t = [None, None]
            ab2 = [None, None]  # (ab2s, ab2o) per layer after finalize

            # ================= emission helpers =================
            def emit_conv_block(L, b):
                wt = wts[L]
                slot0 = slot0s[L]
                r0 = b * R
                if L == 0:
                    xb = xbs[b % 2]
                    if b == 0:
                        nc.vector.memset(xb[0:64, 0:PITCH], 0.0)
                    if b == NB - 1:
                        nc.vector.memset(
                            xb[64:128, (R + 1) * PITCH : (R + 2) * PITCH], 0.0
                        )
                    lo_a = r0 - 1
                    s_a = 0
                    if b == 0:
                        lo_a, s_a = 0, 1
                    n_a = r0 + R - lo_a + 1
                    # split each half's load in two so more DMA engines pull
                    for h0, hn in ((0, n_a // 2), (n_a // 2, n_a - n_a // 2)):
                        nc.sync.dma_start(
                            out=_ap(xb[0:64, :], yoff(s_a + h0),
                                    [[PITCH, hn], [1, W]]),
                            in_=bass.AP(
                                tensor=xh[:].tensor,
                                offset=(lo_a + h0) * W,
                                ap=[[H * W, 64], [W, hn], [1, W]],
                            ),
                        )
                    hb_lo = HH + r0 - 1
                    n_b = R + 2 if b < NB - 1 else R + 1
                    for h0, hn in ((0, n_b // 2), (n_b // 2, n_b - n_b // 2)):
                        nc.sync.dma_start(
                            out=_ap(xb[64:128, :], yoff(h0),
                                    [[PITCH, hn], [1, W]]),
                            in_=bass.AP(
                                tensor=xh[:].tensor,
                                offset=(hb_lo + h0) * W,
                                ap=[[H * W, 64], [W, hn], [1, W]],
                            ),
                        )
                    src_t = xb
                    loc = lambda rr, dy: (rr - r0 + 1 + dy)  # slot in xb
                else:
                    src_t = ysb
                    loc = lambda rr, dy: (rr + dy + 1)       # y1 slot

                # conv: tap-outer, all four PE quadrants concurrently.
                # pts[0]: rows (r0,r0+1)   A->parts 0:64  (q00), B->parts 64:128 (q66)
                # pts[1]: rows (r0+2,r0+3) A->parts 64:128 (q06), B->parts 0:64  (q60)
                pts = [
                    psc.tile([128, 512], f32, tag="cps", name=f"cps_{L}_{b}_{i}")
                    for i in range(R // 2)
                ]
                for t in range(9):
                    dy, dx = t // 3 - 1, t % 3 - 1
                    off0 = yoff(loc(r0, dy)) + dx
                    off1 = yoff(loc(r0 + 2, dy)) + dx
                    nc.tensor.matmul(
                        pts[0][0:64, :], wt[0:64, t, 0:64],
                        _ap(src_t[0:64, :], off0, [[PITCH, 2], [1, W]]),
                        start=(t == 0), stop=(t == 8), tile_position=(0, 0),
                    )
                    nc.tensor.matmul(
                        pts[0][64:128, :], wt[64:128, t, 64:128],
                        _ap(src_t[64:128, :], off0, [[PITCH, 2], [1, W]]),
                        start=(t == 0), stop=(t == 8), tile_position=(64, 64),
                    )
                    nc.tensor.matmul(
                        pts[1][64:128, :], wt[0:64, t, 64:128],
                        _ap(src_t[0:64, :], off1, [[PITCH, 2], [1, W]]),
                        start=(t == 0), stop=(t == 8), tile_position=(0, 64),
                    )
                    nc.tensor.matmul(
                        pts[1][0:64, :], wt[64:128, t, 0:64],
                        _ap(src_t[64:128, :], off1, [[PITCH, 2], [1, W]]),
                        start=(t == 0), stop=(t == 8), tile_position=(64, 0),
                    )
                nc.scalar.copy(
                    out=_ap(ysb[:], yoff(r0 + slot0), [[PITCH, 2], [1, W]]),
                    in_=pts[0][:],
                )
                # pts[1] halves land swapped; cross-partition copies
                nc.scalar.copy(
                    out=_ap(ysb[0:64, :], yoff(r0 + 2 + slot0), [[PITCH, 2], [1, W]]),
                    in_=pts[1][64:128, :],
                )
                nc.vector.tensor_copy(
                    _ap(ysb[64:128, :], yoff(r0 + 2 + slot0), [[PITCH, 2], [1, W]]),
                    pts[1][0:64, :],
                )

            def emit_tp(L, b):
                """Transposes + pixel-major strip + squares for block b."""
                slot0 = slot0s[L]
                r0 = b * R
                pts2 = tpp.tile([128, 1024], f16, tag="tp", name=f"tp_{L}_{b}")
                for j in range(2 * R):
                    rr = r0 + j // 2
                    cs = j % 2
                    src = _ap(ysb[:], yoff(rr + slot0) + cs * 128, [[1, 128]])
                    nc.tensor.transpose(
                        pts2[:, j * 128 : (j + 1) * 128], src, id128sb[:]
                    )
                sp = stripp.tile([128, 2 * R, 256], f16, tag="strip",
                                 name=f"sp_{L}_{b}")
                nc.scalar.copy(
                    out=_ap(sp[:], 0, [[256, 2 * R], [1, 128]]),
                    in_=pts2[:],
                )
                nc.gpsimd.tensor_tensor(
                    _ap(sp[:], 128, [[256, 2 * R], [1, 128]]),
                    _ap(sp[:], 0, [[256, 2 * R], [1, 128]]),
                    _ap(sp[:], 0, [[256, 2 * R], [1, 128]]),
                    ALU.mult,
                )
                return sp

            def emit_stats(L, b, sp):
                stats = stats_t[L]
                for j in range(2 * R):
                    ci = b * 2 * R + j
                    nc.tensor.matmul(
                        stats[:],
                        _ap(maskpm[:], ci * 41, [[1, 41]]),
                        sp[:, j, :],
                        start=(ci == 0), stop=(ci == NST - 1),
                    )

            def emit_finalize(L):
                stats = stats_t[L]
                ssb = smallp.tile([41, 256], f32, tag="ssb", name=f"ssb{L}")
                nc.vector.tensor_copy(ssb[:], stats[:])
                sB = smallp.tile([9, 256], f32, tag="sB", name=f"sB{L}")
                nc.vector.tensor_copy(sB[:], ssb[32:41, :])
                s1 = smallp.tile([9, 64], f32, tag="s1", name=f"s1_{L}")
                s2 = smallp.tile([9, 64], f32, tag="s2", name=f"s2_{L}")
                nc.vector.tensor_tensor(s1[:], ssb[0:9, 0:64], sB[:, 64:128], ALU.add)
                nc.vector.tensor_tensor(s2[:], ssb[0:9, 128:192], sB[:, 192:256], ALU.add)
                mean = smallp.tile([9, 64], f32, tag="mean", name=f"mean{L}")
                nc.vector.tensor_scalar_mul(out=mean[:], in0=s1[:], scalar1=rcsb[:])
                e2 = smallp.tile([9, 64], f32, tag="e2", name=f"e2_{L}")
                nc.vector.tensor_scalar_mul(out=e2[:], in0=s2[:], scalar1=rcsb[:])
                var = smallp.tile([9, 64], f32, tag="var", name=f"var{L}")
                nc.vector.tensor_tensor(var[:], mean[:], mean[:], ALU.mult)
                nc.vector.tensor_tensor(var[:], e2[:], var[:], ALU.subtract)
                sd = smallp.tile([9, 64], f32, tag="sd", name=f"sd{L}")
                nc.scalar.activation(
                    out=sd[:], in_=var[:], func=mybir.ActivationFunctionType.Sqrt,
                    bias=epsap[:], scale=1.0,
                )
                rstd = smallp.tile([9, 64], f32, tag="rstd", name=f"rstd{L}")
                nc.vector.reciprocal(out=rstd[:], in_=sd[:])
                # ab: A at partitions 0:9, B at partitions 32:41
                # gam/bet have their seg-8 (background) row zeroed host-side,
                # so B's row 8 computes to 0; adding bg9 (one-hot row 8) then
                # sets A's row 8 to 1 — no unaligned-partition writes needed.
                ab = smallp.tile([64, 64], f32, tag="ab", name=f"ab{L}")
                nc.vector.tensor_tensor(ab[0:9, :], rstd[:], gam[L][:], ALU.mult)
                mA = smallp.tile([9, 64], f32, tag="mA", name=f"mA{L}")
                nc.vector.tensor_tensor(mA[:], mean[:], ab[0:9, :], ALU.mult)
                nc.vector.tensor_tensor(ab[32:41, :], bet[L][:], mA[:], ALU.subtract)
                nc.vector.tensor_tensor(ab[0:9, :], ab[0:9, :], bg9sb[:], ALU.add)
                # expansion lhsT tiles [18, 128], block-diagonal per half
                ab2s = smallp.tile([41, 128], f16, tag="ab2s", name=f"ab2s{L}")
                ab2o = smallp.tile([41, 128], f16, tag="ab2o", name=f"ab2o{L}")
                nc.vector.memset(ab2s[:], 0.0)
                nc.vector.memset(ab2o[:], 0.0)
                # all placements 32-aligned: A segs rows 0:9, B segs rows 32:41
                nc.vector.tensor_copy(ab2s[0:9, 0:64], ab[0:9, :])
                nc.vector.tensor_copy(ab2s[32:41, 64:128], ab[0:9, :])
                nc.vector.tensor_copy(ab2o[0:9, 0:64], ab[32:41, :])
                nc.vector.tensor_copy(ab2o[32:41, 64:128], ab[32:41, :])
                ab2[L] = (ab2s, ab2o)

            def emit_norm_chunk(L, g, idx):
                slot0 = slot0s[L]
                ab2s, ab2o = ab2[L]
                base = yoff(2 * g + slot0)
                if L == 1:
                    # conv pools are idle in the tail; rotate psum slots
                    # across all three to keep several chunks in flight
                    pool, ptag = ((pse, "exp"), (psc, "cps"), (tpp, "tp"))[idx % 3]
                else:
                    pool, ptag = pse, "exp"
                sE = pool.tile([128, 512], f32, tag=ptag, name=f"se{L}_{g}")
                oE = pool.tile([128, 512], f32, tag=ptag, name=f"oe{L}_{g}")
                win = ms2[0:41, 2 * g * W : (2 * g + 2) * W]
                nc.tensor.matmul(sE[:], ab2s[:], win, start=True, stop=True)
                nc.tensor.matmul(oE[:], ab2o[:], win, start=True, stop=True)
                yv = _ap(ysb[:], base, [[PITCH, 2], [1, W]])
                t1 = normp.tile([128, 512], f16, tag="t1", name=f"t1_{L}_{g}")
                nc.vector.tensor_tensor(t1[:], yv, sE[:], ALU.mult)
                nc.vector.tensor_tensor(t1[:], t1[:], oE[:], ALU.add)
                if L == 0:
                    nc.scalar.activation(
                        out=yv, in_=t1[:],
                        func=mybir.ActivationFunctionType.Relu,
                    )
                    if g == NG - 1:
                        # B halo slot 0 <- normalized A row HH-1 (slot HH)
                        nc.sync.dma_start(
                            out=_ap(ysb[64:128, :], yoff(0), [[1, W]]),
                            in_=_ap(ysb[0:64, :], yoff(HH), [[1, W]]),
                        )
                    if g == 0:
                        # A halo slot HH+1 <- normalized B row 0 (slot 1)
                        nc.sync.dma_start(
                            out=_ap(ysb[0:64, :], yoff(HH + 1), [[1, W]]),
                            in_=_ap(ysb[64:128, :], yoff(1), [[1, W]]),
                        )
                else:
                    st = outp.tile([128, 512], f16, tag="st", name=f"st{g}")
                    nc.scalar.activation(
                        out=st[:], in_=t1[:],
                        func=mybir.ActivationFunctionType.Relu,
                    )
                    for hf in (0, 1):
                        nc.sync.dma_start(
                            out=bass.AP(
                                tensor=out[:].tensor,
                                offset=hf * HW2 + 2 * g * W,
                                ap=[[H * W, 64], [1, 2 * W]],
                            ),
                            in_=st[64 * hf : 64 * hf + 64, :],
                        )

            # ================= layer 0: conv + stats (pipelined) ==========
            stats_t[0] = pss.tile([41, 256], f32, tag="stats", name="stats0")
            sps = {}
            for b in range(NB + 2):
                if b < NB:
                    emit_conv_block(0, b)
                if b == 1:
                    # masks go on the Activation HWDGE ring so they cannot
                    # block the SP ring that feeds the conv input loads
                    nc.scalar.dma_start(out=maskpm[:], in_=maskpm_d[:])
                if b in (6, 10, 14, 18):
                    # ms2 is needed only by the normalize phase; stream its
                    # quarters mid-conv so they never contend with startup
                    q = (b - 6) // 4
                    sl = HW2 // 4
                    nc.scalar.dma_start(
                        out=ms2[:, q * sl : (q + 1) * sl],
                        in_=bass.AP(
                            tensor=ms2_d[:].tensor,
                            offset=q * sl,
                            ap=[[HW2 + 256, 128], [1, sl]],
                        ),
                    )
                if 1 <= b <= NB:
                    sps[b - 1] = emit_tp(0, b - 1)
                if b >= 2:
                    emit_stats(0, b - 2, sps.pop(b - 2))
            emit_finalize(0)

            # ====== interleaved: L0 normalize + L1 conv/stats =============
            stats_t[1] = pss.tile([41, 256], f32, tag="stats", name="stats1")
            order = [NG - 1] + list(range(NG - 1))
            oi = 0
            sps = {}
            for b in range(NB + 2):
                target = min(len(order), 4 + 2 * b)
                while oi < target:
                    emit_norm_chunk(0, order[oi], oi)
                    oi += 1
                if b < NB:
                    emit_conv_block(1, b)
                if 1 <= b <= NB:
                    sps[b - 1] = emit_tp(1, b - 1)
                if b >= 2:
                    emit_stats(1, b - 2, sps.pop(b - 2))
            while oi < len(order):
                emit_norm_chunk(0, order[oi], oi)
                oi += 1
            emit_finalize(1)

            # ================= layer 1 normalize + store ==================
            for g in range(NG):
                emit_norm_chunk(1, g, g)

    return nc


MAXW = 1


def _split_multi_waits(nc):
    """The installed walrus rejects instructions with >MAXW sync waits; hoist
    excess waits onto preceding same-engine nops."""
    nsplit = 0
    for fn in nc.m.functions:
        for blk in fn.blocks:
            insts = list(blk.instructions)
            out = []
            for inst in insts:
                si = inst.sync_info
                waits = list(si.on_wait) if (si and si.on_wait) else []
                if len(waits) > MAXW:
                    for i in range(0, len(waits) - MAXW, MAXW):
                        nop = mybir.InstNoOp(
                            name=f"WSPLIT-{nsplit}", ins=[], outs=[]
                        )
                        nsplit += 1
                        nop.engine = inst.engine
                        nop.sync_info = mybir.SyncInfo(
                            on_wait=waits[i : i + MAXW], on_update=[]
                        )
                        out.append(nop)
                    si.on_wait = waits[len(waits) - MAXW :]
                out.append(inst)
            if len(out) != len(insts):
                while len(blk.instructions):
                    blk.instructions.pop()
                for inst in out:
                    blk.instructions.append(inst)
    return nsplit


def build_nc(H=256, split_waits=True):
    _install_tile_patch()
    nc = bass.Bass()
    emit(nc, H)
    if split_waits:
        n = _split_multi_waits(nc)
        if n:
            print(f"kernel: split {n} multi-wait instructions")
    return nc


# ---------------------------------------------------------------------------
# host-side input prep
# ---------------------------------------------------------------------------
def prep_core_inputs(x_img, ids_img, w0, g0v, b0v, w1, g1v, b1v, H=256):
    """x_img [C,H,W] f32, ids_img [H,W] int -> input map for one core."""
    HH = H // 2
    NST = HH * 2
    seg = np.where(ids_img < 0, 8, ids_img).astype(np.int64)

    m = {}
    m["xh"] = np.ascontiguousarray(x_img.reshape(C, H * W).astype(np.float16))
    cnt = np.bincount(seg.reshape(-1), minlength=9)[:9]
    m["rcnt"] = (1.0 / np.maximum(cnt, 1)).astype(np.float32)

    ids = ids_img.astype(np.int16)
    segv = np.array([0, 1, 2, 3, 4, 5, 6, 7, -1], np.int16)
    # maskpm [128, NST*18]: partition p, chunk ci = rr*2+cs, seg s = 9h+k
    arr = ids.reshape(2, HH, 2, 128)             # [h, rr, cs, p]
    idp = arr.transpose(3, 1, 2, 0)              # [p, rr, cs, h]
    mk = (idp[..., None] == segv)                # [p, rr, cs, h, 9]
    mk18 = mk.reshape(128, NST, 2, 9)            # [p, ci, h, k]
    mk41 = np.zeros((128, NST, 41), np.float16)
    mk41[:, :, 0:9] = mk18[:, :, 0]              # A-half segs
    mk41[:, :, 32:41] = mk18[:, :, 1]            # B-half segs
    m["maskpm"] = np.ascontiguousarray(mk41.reshape(128, NST * 41))
    # ms2 [41, HH*W]: A segs rows 0:9, B segs rows 32:41
    halves = ids.reshape(2, HH * W)              # [h, px]
    msh = halves[:, None, :] == segv[None, :, None]   # [h, 9, px]
    ms41 = np.zeros((128, HH * W + 256), np.float16)
    ms41[0:9, : HH * W] = msh[0]
    ms41[32:41, : HH * W] = msh[1]
    m["ms2"] = np.ascontiguousarray(ms41)

    for name, wmat in (("w0d", w0), ("w1d", w1)):
        wd = np.zeros((9, 128, 128), np.float16)
        for t in range(9):
            dy, dx = t // 3, t % 3
            lhsT = wmat[:, :, dy, dx].T.astype(np.float16)  # [cin, cout]
            wd[t] = np.tile(lhsT, (2, 2))   # all four quadrants
        m[name] = np.ascontiguousarray(wd.transpose(1, 0, 2))  # [ci, t, co]

    m["id128"] = np.eye(128, dtype=np.float16)
    for name, v in (("g0", g0v), ("b0", b0v), ("g1", g1v), ("b1", b1v)):
        tab = np.zeros((9, C), np.float32)
        tab[0:8] = np.asarray(v, np.float32)[None, :]
        m[name] = tab
    bg = np.zeros((9, C), np.float32)
    bg[8] = 1.0
    m["bg9"] = bg
    return m


LAST_RESULT = None


def kernel(features, ins_indices_batch, w0, g0, b0, w1, g1, b1):
    global LAST_RESULT
    _install_ntff_shim()
    from concourse.bass_utils import run_bass_kernel_spmd
    from concourse import bass2jax as _b2j
    import traceback as _tb

    _b2j.install_neuronx_cc_hook()
    import libneuronxla as _lnx

    if not getattr(_lnx, "_ant_dbg_wrapped", False):
        _orig = _lnx.neuronx_cc

        def _dbg(*a, **k):
            try:
                return _orig(*a, **k)
            except BaseException:
                _tb.print_exc()
                raise

        _lnx.neuronx_cc = _dbg
        _lnx._ant_dbg_wrapped = True

    x = np.asarray(features, np.float32)
    ids = np.asarray(ins_indices_batch).astype(np.int64)
    w0 = np.asarray(w0, np.float32)
    w1 = np.asarray(w1, np.float32)
    N = x.shape[0]
    H = x.shape[2]

    nc = build_nc(H)
    in_maps = [
        prep_core_inputs(x[i], ids[i], w0, g0, b0, w1, g1, b1, H) for i in range(N)
    ]
    trace = bool(int(os.environ.get("BASS_KERNEL_TRACE", "0")))
    res = run_bass_kernel_spmd(nc, in_maps, list(range(N)), trace=trace)
    LAST_RESULT = res
    outs = [
        res.results[i]["out"].astype(np.float32).reshape(C, H, W) for i in range(N)
    ]
    return np.stack(outs, 0)
